# revision 7
# baseline (speedup 1.0000x reference)
"""ChildSum TreeLSTM (complete binary trees, heap layout) on 8 Trainium2 cores.

Strategy
--------
Data-parallel over the tree batch: 256 trees -> 32 per core -> processed in
chunks of B_C trees.  All on-chip tensors live in a feature-major
("transposed") layout: partitions = 128-feature chunk (2 chunks cover
D=256), free dim = (node, batch) columns.  The host pre-packs x into this
layout (fp16), so the device never transposes anything and every matmul
contracts over the partition dim directly:

    iou^T[m-chunk, cols] = sum_k W_iou^T[k, m].T @ x^T[k, cols]
                         + sum_k U_iou^T[k, m].T @ h_sum^T[k, cols]

Levels run bottom-up (leaves first); each level is processed in 512-column
blocks: PE fills 2-bank PSUM tiles (i, o, u, f_even, f_odd), ACT applies
sigmoid/tanh out of PSUM into fp16 SBUF, DVE runs the c/h elementwise chain
with even/odd strided views of the child level.  fp16 everywhere on-chip
(PSUM accumulates fp32); fp32 root outputs.
"""

import os
from contextlib import ExitStack

import numpy as np

# Problem constants (hardcoded; kernel.py must be self-contained).
B = 256
DEPTH = 9
NNODES = 2 ** (DEPTH + 1) - 1  # 1023
D = 256
NCORES = 8
B_LOC = B // NCORES            # 32 trees per core
B_C = 4                        # trees per chunk
NCH = B_LOC // B_C             # chunks per core
COLS = NNODES * B_C            # x columns per chunk
XU_NODES = 2 ** 6 - 1          # nodes in levels 0..5 (heap prefix), one DMA

TRACE = False                  # set True (module-level) to request an NTFF trace
LAST = {}                      # info from the most recent run (exec time etc.)

_NC_CACHE = {}


def _build(nch, has_bias):
    """Emit the Bass program for one core processing `nch` chunks."""
    import concourse.mybir as mybir
    import concourse.tile as tile
    from concourse import bacc

    f16 = mybir.dt.float16
    f32 = mybir.dt.float32
    Sig = mybir.ActivationFunctionType.Sigmoid
    Tanh = mybir.ActivationFunctionType.Tanh

    nc = bacc.Bacc(enable_partition_id=False)

    xt = nc.declare_dram_parameter("xt", [nch, 2, 128, COLS], f16, isOutput=False)
    wiou_d = nc.declare_dram_parameter("wiou", [2, 128, 768], f16, isOutput=False)
    uiou_d = nc.declare_dram_parameter("uiou", [2, 128, 768], f16, isOutput=False)
    wf_d = nc.declare_dram_parameter("wf", [2, 128, 256], f16, isOutput=False)
    uf_d = nc.declare_dram_parameter("uf", [2, 128, 256], f16, isOutput=False)
    if has_bias:
        biou_d = nc.declare_dram_parameter("biou", [768], f32, isOutput=False)
        bf_d = nc.declare_dram_parameter("bf", [256], f32, isOutput=False)
    hout = nc.declare_dram_parameter("hout", [2, 128, nch * B_C], f32, isOutput=True)
    cout = nc.declare_dram_parameter("cout", [2, 128, nch * B_C], f32, isOutput=True)

    with tile.TileContext(nc) as tc, ExitStack() as ctx:
        wpool = ctx.enter_context(tc.tile_pool(name="wpool", bufs=1))
        big = ctx.enter_context(tc.tile_pool(name="big", bufs=2))
        trans = ctx.enter_context(tc.tile_pool(name="trans", bufs=3))
        xpool = ctx.enter_context(tc.tile_pool(name="xpool", bufs=2))
        pp = ctx.enter_context(tc.tile_pool(name="pp", bufs=4, space="PSUM"))

        wiou_sb = wpool.tile([128, 2, 768], f16, tag="wiou")
        uiou_sb = wpool.tile([128, 2, 768], f16, tag="uiou")
        wf_sb = wpool.tile([128, 2, 256], f16, tag="wf")
        uf_sb = wpool.tile([128, 2, 256], f16, tag="uf")
        for kc in range(2):
            nc.sync.dma_start(out=wiou_sb[:, kc, :], in_=wiou_d[kc])
            nc.sync.dma_start(out=uiou_sb[:, kc, :], in_=uiou_d[kc])
            nc.sync.dma_start(out=wf_sb[:, kc, :], in_=wf_d[kc])
            nc.sync.dma_start(out=uf_sb[:, kc, :], in_=uf_d[kc])
        if has_bias:
            biou_sb = wpool.tile([128, 6], f32, tag="biou")
            bf_sb = wpool.tile([128, 2], f32, tag="bf")
            for mj in range(6):
                nc.sync.dma_start(
                    out=biou_sb[:, mj : mj + 1],
                    in_=biou_d[mj * 128 : (mj + 1) * 128].rearrange(
                        "(p one) -> p one", one=1
                    ),
                )
            for mj in range(2):
                nc.sync.dma_start(
                    out=bf_sb[:, mj : mj + 1],
                    in_=bf_d[mj * 128 : (mj + 1) * 128].rearrange(
                        "(p one) -> p one", one=1
                    ),
                )

        def act(out_t, in_t, func, bias_sb, bias_cols):
            """Activation; fused across feature chunks when biases are zero."""
            if not has_bias:
                nc.scalar.activation(out=out_t, in_=in_t, func=func)
            else:
                for kk in range(2):
                    col = bias_cols[kk]
                    nc.scalar.activation(
                        out=out_t[:, kk, :],
                        in_=in_t[:, kk, :],
                        func=func,
                        bias=bias_sb[:, col : col + 1],
                    )

        def emit_chunk(ch):
            h_prev = c_prev = None   # level l+1 tensors
            hs_cur = None            # h_sum for the level being processed
            # levels 0..5 of x: one contiguous DMA (heap prefix)
            xu = xpool.tile([128, 2, XU_NODES * B_C], f16, tag="xu")
            for kc in range(2):
                nc.sync.dma_start(
                    out=xu[:, kc, :], in_=xt[ch, kc, :, 0 : XU_NODES * B_C]
                )
            for lvl in range(DEPTH, -1, -1):
                n_l = 1 << lvl
                s_l = n_l - 1
                R = n_l * B_C
                leaf = lvl == DEPTH
                if lvl >= 6:
                    xl = xpool.tile([128, 2, R], f16, tag=f"x{lvl}")
                    for kc in range(2):
                        nc.sync.dma_start(
                            out=xl[:, kc, :],
                            in_=xt[ch, kc, :, s_l * B_C : (s_l + n_l) * B_C],
                        )
                    xoff = 0
                else:
                    xl = xu
                    xoff = s_l * B_C

                h_l = big.tile([128, 2, R], f16, tag=f"h{lvl % 2}", name=f"h_l{lvl}")
                c_l = big.tile([128, 2, R], f16, tag=f"c{lvl % 2}", name=f"c_l{lvl}")
                hs_next = None
                if lvl > 0:
                    hs_next = big.tile(
                        [128, 2, R // 2], f16, tag=f"s{(lvl - 1) % 2}",
                        name=f"hs_l{lvl - 1}",
                    )

                P = min(R, 512)
                for blk in range(R // P):
                    bo = blk * P
                    nq = P // B_C

                    def mm_group(mcol_base, w_sb, extra):
                        """PSUM tile [128,2,P]; slice mj accumulates
                        w_sb[:,kc,(mcol_base+mj)*128...] terms + extras."""
                        pt = pp.tile([128, 2, P], f32, tag="ps", name="pt")
                        for mj in range(2):
                            terms = []
                            for kc in range(2):
                                terms.append(
                                    (w_sb[:, kc, (mcol_base + mj) * 128 :
                                          (mcol_base + mj + 1) * 128],
                                     xl[:, kc, xoff + bo : xoff + bo + P])
                                )
                            for (lw, lrhs) in extra(mj):
                                terms.append((lw, lrhs))
                            for t_i, (lw, lrhs) in enumerate(terms):
                                nc.tensor.matmul(
                                    pt[:, mj, :], lw, lrhs,
                                    start=(t_i == 0),
                                    stop=(t_i == len(terms) - 1),
                                )
                        return pt

                    if leaf:
                        no_extra = lambda mj: []
                        i_ps = mm_group(0, wiou_sb, no_extra)
                        o_ps = mm_group(2, wiou_sb, no_extra)
                        u_ps = mm_group(4, wiou_sb, no_extra)
                    else:
                        def iou_extra(base):
                            def ex(mj):
                                return [
                                    (uiou_sb[:, kc, (base + mj) * 128 :
                                             (base + mj + 1) * 128],
                                     hs_cur[:, kc, bo : bo + P])
                                    for kc in range(2)
                                ]
                            return ex

                        i_ps = mm_group(0, wiou_sb, iou_extra(0))
                        o_ps = mm_group(2, wiou_sb, iou_extra(2))
                        u_ps = mm_group(4, wiou_sb, iou_extra(4))

                        def child_view(t, kc, par):
                            v = t[:, kc, 2 * bo : 2 * bo + 2 * P].rearrange(
                                "p (q two b) -> p q two b", two=2, b=B_C
                            )
                            return v[:, :, par, :]

                        def f_extra(par):
                            def ex(mj):
                                return [
                                    (uf_sb[:, kc, mj * 128 : (mj + 1) * 128],
                                     child_view(h_prev, kc, par))
                                    for kc in range(2)
                                ]
                            return ex

                        fe_ps = mm_group(0, wf_sb, f_extra(0))
                        fo_ps = mm_group(0, wf_sb, f_extra(1))

                    i_sb = trans.tile([128, 2, P], f16, tag="isb", name="i_sb")
                    o_sb = trans.tile([128, 2, P], f16, tag="osb", name="o_sb")
                    u_sb = trans.tile([128, 2, P], f16, tag="usb", name="u_sb")
                    act(i_sb, i_ps, Sig, biou_sb if has_bias else None, (0, 1))
                    act(o_sb, o_ps, Sig, biou_sb if has_bias else None, (2, 3))
                    act(u_sb, u_ps, Tanh, biou_sb if has_bias else None, (4, 5))

                    c_blk = c_l[:, :, bo : bo + P]
                    nc.vector.tensor_mul(c_blk, i_sb, u_sb)

                    if not leaf:
                        fe_sb = trans.tile([128, 2, P], f16, tag="fesb", name="fe_sb")
                        fo_sb = trans.tile([128, 2, P], f16, tag="fosb", name="fo_sb")
                        act(fe_sb, fe_ps, Sig, bf_sb if has_bias else None, (0, 1))
                        act(fo_sb, fo_ps, Sig, bf_sb if has_bias else None, (0, 1))
                        tm_e = trans.tile([128, 2, P], f16, tag="tme", name="tm_e")
                        tm_o = trans.tile([128, 2, P], f16, tag="tmo", name="tm_o")
                        for par, f_sb, tm in ((0, fe_sb, tm_e), (1, fo_sb, tm_o)):
                            for kk in range(2):
                                fv = f_sb[:, kk, :].rearrange(
                                    "p (q b) -> p q b", b=B_C
                                )
                                tv = tm[:, kk, :].rearrange("p (q b) -> p q b", b=B_C)
                                cv = child_view(c_prev, kk, par)
                                nc.vector.tensor_mul(tv, fv, cv)
                        nc.vector.tensor_add(c_blk, c_blk, tm_e)
                        nc.vector.tensor_add(c_blk, c_blk, tm_o)

                    t_sb = trans.tile([128, 2, P], f16, tag="tsb", name="t_sb")
                    nc.scalar.activation(out=t_sb, in_=c_blk, func=Tanh)
                    h_blk = h_l[:, :, bo : bo + P]
                    nc.vector.tensor_mul(h_blk, o_sb, t_sb)

                    if lvl > 0:
                        for kk in range(2):
                            hv = h_l[:, kk, bo : bo + P].rearrange(
                                "p (q two b) -> p q two b", two=2, b=B_C
                            )
                            sv = hs_next[:, kk, bo // 2 : bo // 2 + P // 2].rearrange(
                                "p (q b) -> p q b", b=B_C
                            )
                            nc.vector.tensor_add(sv, hv[:, :, 0, :], hv[:, :, 1, :])

                h_prev, c_prev = h_l, c_l
                hs_cur = hs_next

            h32 = trans.tile([128, 2, B_C], f32, tag="h32", name="h32")
            c32 = trans.tile([128, 2, B_C], f32, tag="c32", name="c32")
            nc.vector.tensor_copy(out=h32, in_=h_prev)
            nc.vector.tensor_copy(out=c32, in_=c_prev)
            for kc in range(2):
                nc.sync.dma_start(
                    out=hout[kc][:, ch * B_C : (ch + 1) * B_C], in_=h32[:, kc, :]
                )
                nc.sync.dma_start(
                    out=cout[kc][:, ch * B_C : (ch + 1) * B_C], in_=c32[:, kc, :]
                )

        for ch in range(nch):
            emit_chunk(ch)

    nc.compile()
    return nc


def _get_nc(nch, has_bias):
    key = (nch, has_bias)
    if key not in _NC_CACHE:
        _NC_CACHE[key] = _build(nch, has_bias)
    return _NC_CACHE[key]


def _pack_inputs(x, W_iou, b_iou, U_iou, W_f, b_f, U_f, nch=NCH):
    """Host-side shard + layout prep. Returns (in_maps, has_bias)."""
    x = np.asarray(x, dtype=np.float32)
    # [core, ch, b, node, d] -> [core, ch, d, node, b]
    xt = x.reshape(NCORES, NCH, B_C, NNODES, D)
    xt = np.ascontiguousarray(
        xt.transpose(0, 1, 4, 3, 2), dtype=np.float16
    ).reshape(NCORES, NCH, 2, 128, COLS)

    wiou = np.ascontiguousarray(
        np.asarray(W_iou, np.float32).T, dtype=np.float16
    ).reshape(2, 128, 768)
    uiou = np.ascontiguousarray(
        np.asarray(U_iou, np.float32).T, dtype=np.float16
    ).reshape(2, 128, 768)
    wf = np.ascontiguousarray(
        np.asarray(W_f, np.float32).T, dtype=np.float16
    ).reshape(2, 128, 256)
    uf = np.ascontiguousarray(
        np.asarray(U_f, np.float32).T, dtype=np.float16
    ).reshape(2, 128, 256)

    b_iou = np.asarray(b_iou, np.float32)
    b_f = np.asarray(b_f, np.float32)
    has_bias = bool(np.any(b_iou) or np.any(b_f))

    in_maps = []
    for c in range(NCORES):
        m = {
            "xt": np.ascontiguousarray(xt[c, :nch]),
            "wiou": wiou,
            "uiou": uiou,
            "wf": wf,
            "uf": uf,
        }
        if has_bias:
            m["biou"] = b_iou
            m["bf"] = b_f
        in_maps.append(m)
    return in_maps, has_bias


class _PjrtRunner:
    """Persistent-jit SPMD executor for a Bass program over 8 neuron devices.

    Mirrors concourse.bass2jax.run_bass_via_pjrt's multi-core branch, but
    keeps the compiled executable and device-resident inputs across calls so
    repeated executions (and timing runs) don't recompile or re-upload.
    """

    def __init__(self, nc):
        import jax
        import concourse.mybir as mybir
        from concourse.bass2jax import _bass_exec_p, install_neuronx_cc_hook
        from jax.sharding import Mesh, NamedSharding, PartitionSpec
        from jax.experimental.shard_map import shard_map

        install_neuronx_cc_hook()
        assert nc.partition_id_tensor is None

        self.jax = jax
        in_names, out_names, out_avals = [], [], []
        for alloc in nc.m.functions[0].allocations:
            if not isinstance(alloc, mybir.MemoryLocationSet):
                continue
            name = alloc.memorylocations[0].name
            if alloc.kind == "ExternalInput":
                in_names.append(name)
            elif alloc.kind == "ExternalOutput":
                out_names.append(name)
                out_avals.append(
                    jax.core.ShapedArray(
                        tuple(alloc.tensor_shape), mybir.dt.np(alloc.dtype)
                    )
                )
        self.in_names, self.out_names, self.out_avals = in_names, out_names, out_avals
        n_params = len(in_names)
        n_outs = len(out_names)
        all_in = in_names + out_names

        def _body(*args):
            return tuple(
                _bass_exec_p.bind(
                    *args,
                    out_avals=tuple(out_avals),
                    in_names=tuple(all_in),
                    out_names=tuple(out_names),
                    lowering_input_output_aliases=(),
                    sim_require_finite=True,
                    sim_require_nnan=True,
                    nc=nc,
                )
            )

        devices = jax.devices()[:NCORES]
        self.mesh = Mesh(np.asarray(devices), ("core",))
        spec = PartitionSpec("core")
        self.sharding = NamedSharding(self.mesh, spec)
        donate = tuple(range(n_params, n_params + n_outs))
        self.fn = jax.jit(
            shard_map(
                _body,
                mesh=self.mesh,
                in_specs=(spec,) * (n_params + n_outs),
                out_specs=(spec,) * n_outs,
                check_rep=False,
            ),
            donate_argnums=donate,
            keep_unused=True,
        )
        self.dev_inputs = None

    def put_inputs(self, in_maps):
        jax = self.jax
        concat = [
            np.concatenate([np.asarray(m[nm]) for m in in_maps], axis=0)
            for nm in self.in_names
        ]
        self.dev_inputs = [jax.device_put(a, self.sharding) for a in concat]
        for a in self.dev_inputs:
            a.block_until_ready()

    def _zero_outs(self):
        jax = self.jax
        zs = [
            jax.device_put(
                np.zeros((NCORES * av.shape[0], *av.shape[1:]), av.dtype),
                self.sharding,
            )
            for av in self.out_avals
        ]
        for z in zs:
            z.block_until_ready()
        return zs

    def run(self):
        outs = self.fn(*self.dev_inputs, *self._zero_outs())
        return {
            nm: np.asarray(outs[i]).reshape(NCORES, *self.out_avals[i].shape)
            for i, nm in enumerate(self.out_names)
        }

    def time_runs(self, n=5):
        import time

        times = []
        for _ in range(n):
            zs = self._zero_outs()
            t0 = time.perf_counter()
            outs = self.fn(*self.dev_inputs, *zs)
            for o in outs:
                o.block_until_ready()
            times.append(time.perf_counter() - t0)
        return times


_RUNNERS = {}


def _get_runner(nch, has_bias):
    key = (nch, has_bias)
    if key not in _RUNNERS:
        _RUNNERS[key] = _PjrtRunner(_get_nc(nch, has_bias))
    return _RUNNERS[key]


def kernel(x, W_iou, b_iou, U_iou, W_f, b_f, U_f):
    in_maps, has_bias = _pack_inputs(x, W_iou, b_iou, U_iou, W_f, b_f, U_f)
    runner = _get_runner(NCH, has_bias)
    runner.put_inputs(in_maps)
    res = runner.run()
    LAST["runner"] = runner

    h = np.empty((B, D), np.float32)
    c = np.empty((B, D), np.float32)
    for i in range(NCORES):
        h[i * B_LOC : (i + 1) * B_LOC] = res["hout"][i].reshape(D, B_LOC).T
        c[i * B_LOC : (i + 1) * B_LOC] = res["cout"][i].reshape(D, B_LOC).T
    return h, c
